# revision 66
# baseline (speedup 1.0000x reference)
"""Trainium2 Bass kernel v5 for per-(batch,channel) circular autocorrelation:

    out = ifft2(|fft2(img - mean(img))|^2).real / (H*W)

Dense-DFT formulation (no FFT primitive on TRN2); per 256x256 image:
  s1: [U 129 | V'' 127] = d^T [CoH | SiH'']   4 bf16 matmuls, FD 256.
      V(j=0) = V(j=128) = 0 identically, so only j=1..127 of V is stored --
      the whole s1 output packs into ONE PSUM bank. Input d arrives bf16
      (host converts; halves input HBM traffic; rel-err budget is ample).
  s2: G1 = Co U - Si V ; G2 = Si U + Co V    16 bf16 matmuls, 2048 cols.
      V-contributions are FD-127 sub-region accumulates (j=1..127).
  P:  P = G1^2 + G2^2 (bf16), DC bin zeroed (== mean subtraction)
  s4a: Q = P^T [Co|Si] over x=0..128 ONLY (Qr x-symmetric, Qi
       x-antisymmetric), rows j=128..1; the DC (j=0) cosine row Qr0 is
       rank-1 accumulated into the Nyquist Qi slot (exactly 0 by symmetry).
  s4b: out[y,:] = cw'^T Qr + sw'^T Qi'   8 bf16 matmuls, 1024 cols:
       direct x=0..128 (FD 129) plus mirrored x=129..255 (FD 127) streamed
       via reversed-stride rhs with sign-folded weights (sw2) -- no DVE
       mirror copies. cw' rows j=128..1; sw'/sw2 row 0 := s absorbs the DC
       term. Output staged bf16 (halves output HBM traffic; host upcasts).

~4.87k PE col-cycles/image (~2.03us); 4-stage software pipeline at stage
distance 2. Squares on ACT, UV copy on DVE, P-add on GpSimd (DC bin dropped
via zeroed-row const cshdc, no memset), o-copy split DVE/ACT, all DMAs on
the SP (HWDGE) queue. PSUM: 8 banks = p1(1) p2(2)x2bufs p4(1) p5(1)x2bufs.
Timing programs use For_i(staggered_reset=True) with TWO passes per body:
no full drain+sem-reset barrier per pass (was ~30us), stage barriers
amortized over 2 passes.

Sharding: pure data parallel, 8 batches per core (64 images of 64x8 b/c).
"""

import numpy as np

N = 256
J = N // 2 + 1  # 129
B, H, W, C = 64, 256, 256, 8
N_CORES = 8
IMGS_PER_CORE = (B // N_CORES) * C  # 64


def _make_consts():
    import ml_dtypes

    bf = ml_dtypes.bfloat16
    a = np.arange(N, dtype=np.float64)
    # half-spectrum stored j-REVERSED: U column c <-> j = 128 - c, so that
    # the special rows land on partition 0 downstream (matmul output base
    # partition must be 0/32/64). V'' column 129+t <-> j = 127 - t.
    j = np.arange(128, -1, -1, dtype=np.float64)  # 128..0
    jv = np.arange(127, 0, -1, dtype=np.float64)  # 127..1 (V'' columns)
    ang = 2.0 * np.pi / N

    CoH = np.cos(ang * np.outer(a, j))  # [256, 129]
    SiHv = np.sin(ang * np.outer(a, jv))  # [256, 127]
    cosih2 = np.concatenate([CoH, SiHv], axis=1).astype(bf)  # [256, 256]

    Co = np.cos(ang * np.outer(a, a))
    Si = np.sin(ang * np.outer(a, a))
    cosi_bf = np.concatenate([Co, Si], axis=1).astype(bf)  # [256, 512]
    nsi_bf = (-Si).astype(bf)  # [256, 256]
    # s4a rhs: only x = 0..128 needed (Qr symmetric / Qi antisymmetric in x)
    csh_bf = np.concatenate([Co[:, 0:129], Si[:, 0:129]], axis=1).astype(bf)
    # DC rank-1 variant with row k=0 zeroed: drops the P[0,0] (DC-DC) bin,
    # which implements the mean subtraction without a per-image memset
    csh0_dc = Co[:, 0:129].copy()
    csh0_dc[0, :] = 0.0
    csh_dc_bf = csh0_dc.astype(bf)

    s = 1.0 / float(N) ** 4
    jm = np.arange(128, 0, -1, dtype=np.float64)  # j = 128..1 (row m <-> 128-m)
    w = np.full(128, 2.0)
    w[0] = 1.0  # j=128 (Nyquist) counted once
    cw = (s * w[:, None] * np.cos(ang * np.outer(jm, a))).astype(bf)  # [128, 256]
    swneg64 = -s * w[:, None] * np.sin(ang * np.outer(jm, a))
    swneg64[0, :] = s  # DC row (j=0, w=1, cos(0)=1): y-independent
    swneg = swneg64.astype(bf)
    # mirrored-x half weights: Qi is x-antisymmetric (negate sin rows), but
    # row 0 carries the smuggled x-SYMMETRIC DC row Qr0 (keep its sign)
    sw2_64 = -swneg64
    sw2_64[0, :] = swneg64[0, :]
    sw2 = sw2_64.astype(bf)

    return dict(
        cosih2=np.ascontiguousarray(cosih2),
        cosi_bf=np.ascontiguousarray(cosi_bf),
        nsi_bf=np.ascontiguousarray(nsi_bf),
        csh_bf=np.ascontiguousarray(csh_bf),
        csh_dc=np.ascontiguousarray(csh_dc_bf),
        cw=np.ascontiguousarray(cw),
        swneg=np.ascontiguousarray(swneg),
        sw2=np.ascontiguousarray(sw2),
    )


def build_program(n_imgs=IMGS_PER_CORE, n_cores=N_CORES, n_iter=1,
                  fake_shared_weights=False, mode="full"):
    """Build the Bass/Tile program. Returns nc.

    n_iter > 1 wraps the whole image pipeline in a For_i hardware loop that
    repeats it n_iter times (same inputs/outputs each pass) — used only by
    the timing harness to measure per-pass device time without compiling a
    huge unrolled program.

    fake_shared_weights / mode are timing-experiment knobs (numerically
    wrong outputs): mode in {"full", "dmaonly", "dmaonly2", "nodma"}.
    """
    from contextlib import ExitStack

    import concourse.bacc as bacc
    import concourse.tile as tile
    from concourse import mybir

    f32 = mybir.dt.float32
    bf16 = mybir.dt.bfloat16

    nc = bacc.Bacc(
        "TRN2",
        target_bir_lowering=False,
        debug=False,
        num_devices=n_cores,
    )

    x_d = nc.dram_tensor("x", [n_imgs, N, N], bf16, kind="ExternalInput").ap()
    cosih2_d = nc.dram_tensor("cosih2", [N, 256], bf16, kind="ExternalInput").ap()
    cosi_d = nc.dram_tensor("cosi_bf", [N, 512], bf16, kind="ExternalInput").ap()
    nsi_d = nc.dram_tensor("nsi_bf", [N, 256], bf16, kind="ExternalInput").ap()
    csh_d = nc.dram_tensor("csh_bf", [N, 258], bf16, kind="ExternalInput").ap()
    cshdc_d = nc.dram_tensor("csh_dc", [N, 129], bf16, kind="ExternalInput").ap()
    cw_d = nc.dram_tensor("cw", [128, N], bf16, kind="ExternalInput").ap()
    swneg_d = nc.dram_tensor("swneg", [128, N], bf16, kind="ExternalInput").ap()
    sw2_d = nc.dram_tensor("sw2", [128, N], bf16, kind="ExternalInput").ap()
    out_d = nc.dram_tensor("out", [n_imgs, N, N], bf16, kind="ExternalOutput").ap()

    with tile.TileContext(nc) as tc, ExitStack() as ctx:
        singles = ctx.enter_context(tc.tile_pool(name="singles", bufs=1))
        dpool = ctx.enter_context(tc.tile_pool(name="dpool", bufs=4))
        uvpool = ctx.enter_context(tc.tile_pool(name="uvpool", bufs=4))
        ppool = ctx.enter_context(tc.tile_pool(name="ppool", bufs=4))
        tpool = ctx.enter_context(tc.tile_pool(name="tpool", bufs=6))
        qpool = ctx.enter_context(tc.tile_pool(name="qpool", bufs=4))
        opool = ctx.enter_context(tc.tile_pool(name="opool", bufs=4))
        ps1 = ctx.enter_context(tc.tile_pool(name="ps1", bufs=1, space="PSUM"))
        ps2 = ctx.enter_context(tc.tile_pool(name="ps2", bufs=2, space="PSUM"))
        ps4 = ctx.enter_context(tc.tile_pool(name="ps4", bufs=1, space="PSUM"))
        ps5 = ctx.enter_context(tc.tile_pool(name="ps5", bufs=2, space="PSUM"))

        # --- constants into SBUF ---
        cosih2 = [singles.tile([128, 256], bf16, tag=f"cosih2{h}", name=f"cosih2{h}") for h in range(2)]
        cosib = [singles.tile([128, 512], bf16, tag=f"cosib{h}", name=f"cosib{h}") for h in range(2)]
        nsib = [singles.tile([128, 256], bf16, tag=f"nsib{h}", name=f"nsib{h}") for h in range(2)]
        cshb = [singles.tile([128, 258], bf16, tag=f"cshb{h}", name=f"cshb{h}") for h in range(2)]
        cshdc = singles.tile([128, 129], bf16, tag="cshdc", name="cshdc")
        nc.sync.dma_start(out=cshdc, in_=cshdc_d[0:128, :])
        for h in range(2):
            sl = slice(128 * h, 128 * (h + 1))
            nc.sync.dma_start(out=cosih2[h], in_=cosih2_d[sl, :])
            nc.sync.dma_start(out=cosib[h], in_=cosi_d[sl, :])
            nc.sync.dma_start(out=nsib[h], in_=nsi_d[sl, :])
            nc.sync.dma_start(out=cshb[h], in_=csh_d[sl, :])
        cw = singles.tile([128, N], bf16, tag="cw", name="cw")
        swneg = singles.tile([128, N], bf16, tag="swneg", name="swneg")
        sw2 = singles.tile([128, N], bf16, tag="sw2", name="sw2")
        nc.sync.dma_start(out=cw, in_=cw_d)
        nc.sync.dma_start(out=swneg, in_=swneg_d)
        nc.sync.dma_start(out=sw2, in_=sw2_d)

        # ablation-mode scratch tiles (timing experiments only)
        if mode != "full":
            dshare = singles.tile([128, 2, N], bf16, tag="dsh", name="dsh")
            osrc = singles.tile([128, 512], bf16, tag="osrc", name="osrc")
            nc.gpsimd.memset(dshare, 0.001)
            nc.gpsimd.memset(osrc, 0.0)

        mm0 = nc.tensor.matmul

        if fake_shared_weights == "split":
            # timing probe: split every matmul into two half-FD matmuls
            # (numerically exact; doubles instruction/ldweights count)
            def mm(out, lhsT, rhs, start, stop):
                n = rhs.free_size()
                if n < 2:
                    return mm0(out, lhsT, rhs, start=start, stop=stop)
                h = n // 2
                mm0(out[:, 0:h], lhsT, rhs[:, 0:h], start=start, stop=stop)
                mm0(out[:, h:n], lhsT, rhs[:, h:n], start=start, stop=stop)
        elif fake_shared_weights:
            def mm(out, lhsT, rhs, **kw):
                if lhsT.dtype == bf16 and lhsT.free_size() == 128:
                    lhsT = cosib[0][:, 0:128]
                return mm0(out, lhsT, rhs, **kw)
        else:
            mm = mm0

        st = {}

        def stageA(i):
            # load + s1: [U | V''] = d^T [CoH | SiH'']  (bf16, FD=256)
            if mode == "nodma":
                d = dshare
            else:
                d = dpool.tile([128, 2, N], bf16, tag="d", name="d")
                nc.sync.dma_start(
                    out=d,
                    in_=x_d[i].rearrange("(h p) c -> p h c", h=2),
                )
            if mode.startswith("dmaonly"):
                return
            p1 = ps1.tile([128, 2, 256], f32, tag="s1", name="s1")
            for xh in range(2):
                xs = slice(128 * xh, 128 * (xh + 1))
                mm(p1[:, xh, :], d[:, 0, xs], cosih2[0], start=True, stop=False)
                mm(p1[:, xh, :], d[:, 1, xs], cosih2[1], start=False, stop=True)
            uv = uvpool.tile([128, 2, 256], bf16, tag="uv", name="uv")
            nc.vector.tensor_copy(out=uv, in_=p1)
            st[i] = {"uv": uv}

        def stageB(i):
            # s2: G1 = Co U - Si V ; G2 = Si U + Co V  (bf16), then
            # P = G1^2 + G2^2 with DC bin zeroed.
            # G1 at p2[:,kt,0:129], G2 at p2[:,kt,129:258], both j-reversed;
            # V'' terms are FD-127 sub-region accumulates (j = 127..1).
            if mode.startswith("dmaonly"):
                return
            uv = st[i]["uv"]
            # 4-product s2: G1 = CoU - SiV at p2[:,kt,0:129], G2 = SiU + CoV
            # at p2[:,kt,129:258]; V'' terms are FD-127 sub-region
            # accumulates (j=127..1). Groups kept sequential per bank.
            p2 = ps2.tile([128, 2, 512], f32, tag="s2", name="s2")
            for kt in range(2):
                ks = slice(128 * kt, 128 * (kt + 1))
                ss = slice(256 + 128 * kt, 256 + 128 * (kt + 1))
                g1 = p2[:, kt, 0:129]
                g1v = p2[:, kt, 1:128]
                g2 = p2[:, kt, 129:258]
                g2v = p2[:, kt, 130:257]
                u0, u1 = uv[:, 0, 0:129], uv[:, 1, 0:129]
                v0, v1 = uv[:, 0, 129:256], uv[:, 1, 129:256]
                mm(g1, cosib[0][:, ks], u0, start=True, stop=False)
                mm(g1v, nsib[0][:, ks], v0, start=False, stop=False)
                mm(g1, cosib[1][:, ks], u1, start=False, stop=False)
                mm(g1v, nsib[1][:, ks], v1, start=False, stop=True)
                mm(g2, cosib[0][:, ss], u0, start=True, stop=False)
                mm(g2v, cosib[0][:, ks], v0, start=False, stop=False)
                mm(g2, cosib[1][:, ss], u1, start=False, stop=False)
                mm(g2v, cosib[1][:, ks], v1, start=False, stop=True)
            tsq = tpool.tile([128, 2, 258], bf16, tag="tsq", name="tsq")
            nc.scalar.activation(out=tsq, in_=p2[:, :, 0:258],
                                 func=mybir.ActivationFunctionType.Square)
            P = ppool.tile([128, 2, 129], bf16, tag="P", name="P")
            # SBUF-only elementwise add on the otherwise-idle GpSimd engine.
            # (The DC bin P[k=0,j=0] is dropped later via the zeroed row in
            # cshdc -- no per-image memset needed.)
            nc.gpsimd.tensor_add(P, tsq[:, :, 0:129], tsq[:, :, 129:258])
            st[i]["P"] = P

        def stageC(i):
            if mode.startswith("dmaonly"):
                return
            # s4a over x = 0..128 only: Qr is x-symmetric, Qi x-antisymmetric.
            # p4[m, 0:129] = Qr, p4[m, 129:258] = Qi, rows j = 128-m (m=0 is
            # the Nyquist row, whose Qi is ~0 by symmetry); the DC (j=0)
            # cosine row Qr0 is rank-1 accumulated into that slot.
            P = st[i]["P"]
            p4 = ps4.tile([128, 258], f32, tag="s4a", name="s4a")
            # cshdc = cos rows with k=0 zeroed: drops the P[0,0] DC-DC bin
            mm(p4, P[:, 0, 0:128], cshb[0], start=True, stop=False)
            mm(p4[0:1, 129:258], P[:, 0, 128:129], cshdc,
               start=False, stop=False)
            mm(p4[0:1, 129:258], P[:, 1, 128:129], cshb[1][:, 0:129],
               start=False, stop=False)
            mm(p4, P[:, 1, 0:128], cshb[1], start=False, stop=True)
            # half-x [Qr | Qi] only; the x>=129 mirror happens for free in
            # stageD via reversed-stride matmul rhs + sign-folded weights
            qrqi = qpool.tile([128, 2, 129], bf16, tag="qrqi", name="qrqi")
            nc.scalar.activation(out=qrqi,
                                 in_=p4.rearrange("p (b c) -> p b c", b=2),
                                 func=mybir.ActivationFunctionType.Copy)
            st[i]["qrqi"] = qrqi

        def stageD(i):
            if mode.startswith("dmaonly"):
                eng = nc.gpsimd if mode == "dmaonly2" else nc.sync
                eng.dma_start(
                    out=out_d[i].rearrange("(h p) c -> p h c", h=2),
                    in_=osrc.rearrange("p (h c) -> p h c", h=2),
                )
                return
            # s4b: y rows 0..127 (cols 0:256) and 128..255 (cols 256:512).
            # x split: direct x=0..128 (FD 129) + mirrored x=129..255 (FD 127)
            # read via reversed-stride rhs; Qi sign flip folded into sw2.
            # swneg/sw2 row 0 carries the DC term (see _make_consts).
            qrqi = st[i]["qrqi"]
            qr = qrqi[:, 0, :]
            qi = qrqi[:, 1, :]
            qr_rev = qrqi[:, 0, 127:0:-1]
            qi_rev = qrqi[:, 1, 127:0:-1]
            p5 = ps5.tile([128, 512], f32, tag="s4b", name="s4b")
            for yh, c0 in ((0, 0), (1, 256)):
                ws = slice(128 * yh, 128 * (yh + 1))
                mm(p5[:, c0:c0 + 129], cw[:, ws], qr, start=True, stop=False)
                mm(p5[:, c0:c0 + 129], swneg[:, ws], qi, start=False, stop=True)
                mm(p5[:, c0 + 129:c0 + 256], cw[:, ws], qr_rev,
                   start=True, stop=False)
                mm(p5[:, c0 + 129:c0 + 256], sw2[:, ws], qi_rev,
                   start=False, stop=True)
            # evacuate PSUM->SBUF (bf16) split across DVE/ACT
            o = opool.tile([128, 512], bf16, tag="o", name="o")
            nc.vector.tensor_copy(out=o[:, 0:256], in_=p5[:, 0:256])
            nc.scalar.activation(out=o[:, 256:512], in_=p5[:, 256:512],
                                 func=mybir.ActivationFunctionType.Copy)
            if mode != "nodma":
                nc.sync.dma_start(
                    out=out_d[i].rearrange("(h p) c -> p h c", h=2),
                    in_=o.rearrange("p (h c) -> p h c", h=2),
                )
            del st[i]

        # software pipeline, stage distance 2: producers get a full extra
        # tick of slack before their consumer stage runs. Deepest stage
        # first, so no engine's stream blocks on a same-image downstream dep.
        def pipeline():
            for t in range(n_imgs + 6):
                if 0 <= t - 6 < n_imgs:
                    stageD(t - 6)
                if 0 <= t - 4 < n_imgs:
                    stageC(t - 4)
                if 0 <= t - 2 < n_imgs:
                    stageB(t - 2)
                if t < n_imgs:
                    stageA(t)

        all_eng = list(mybir.ALL_ENGINES)
        if n_iter == 1:
            pipeline()
        elif n_iter % 8 == 0:
            # eight passes per body: stage-barrier overhead amortized 8x
            with tc.For_i(0, n_iter // 8, 1, staggered_reset=True,
                          hint_engines=all_eng):
                for _ in range(8):
                    pipeline()
        elif n_iter % 4 == 0:
            # staggered_reset: no full drain + semaphore-reset barrier on the
            # loop back-edge; four passes per body amortize the per-iteration
            # stage-barrier overhead; back-edge branch-prefetch hints cut
            # instruction-fetch stalls at the body top.
            with tc.For_i(0, n_iter // 4, 1, staggered_reset=True,
                          hint_engines=all_eng):
                for _ in range(4):
                    pipeline()
        elif n_iter % 2 == 0:
            with tc.For_i(0, n_iter // 2, 1, staggered_reset=True,
                          hint_engines=all_eng):
                pipeline()
                pipeline()
        else:
            with tc.For_i(0, n_iter, 1, staggered_reset=True,
                          hint_engines=all_eng):
                pipeline()

    nc.compile()
    return nc


_CACHED = {}


def _get_program(n_imgs, n_cores):
    key = (n_imgs, n_cores)
    if key not in _CACHED:
        _CACHED[key] = build_program(n_imgs, n_cores)
    return _CACHED[key]


def make_in_maps(x):
    """Shard + bf16-convert full [B,H,W,C] f32 input into per-core maps."""
    import ml_dtypes

    consts = _make_consts()
    bpc = B // N_CORES
    xb = x.astype(ml_dtypes.bfloat16)
    in_maps = []
    for core in range(N_CORES):
        shard = xb[core * bpc:(core + 1) * bpc]  # [8, 256, 256, 8] bf16
        shard = np.ascontiguousarray(shard.transpose(0, 3, 1, 2)).reshape(
            IMGS_PER_CORE, H, W
        )
        m = {"x": shard}
        m.update(consts)
        in_maps.append(m)
    return in_maps


def kernel(inputs: np.ndarray) -> np.ndarray:
    """inputs: [64, 256, 256, 8] float32 -> output same shape."""
    from concourse.bass_utils import run_bass_kernel_spmd

    inputs = np.asarray(inputs, dtype=np.float32)
    assert inputs.shape == (B, H, W, C)

    nc = _get_program(IMGS_PER_CORE, N_CORES)
    in_maps = make_in_maps(inputs)

    res = run_bass_kernel_spmd(nc, in_maps, core_ids=list(range(N_CORES)))

    bpc = B // N_CORES
    out = np.empty((B, H, W, C), dtype=np.float32)
    for core in range(N_CORES):
        o = np.asarray(res.results[core]["out"], dtype=np.float32)
        o = o.reshape(bpc, C, H, W)
        out[core * bpc:(core + 1) * bpc] = o.transpose(0, 2, 3, 1)
    return out


if __name__ == "__main__":
    rng = np.random.default_rng(0)
    x = rng.standard_normal((B, H, W, C)).astype(np.float32)
    y = kernel(x)
    print("kernel output:", y.shape, y.dtype)
